# revision 1
# baseline (speedup 1.0000x reference)
"""Multi-head attention (B=16, C=256, N=1024, H=4 heads) on 8 TRN2 NeuronCores.

Data-parallel over batch: 2 images per core, weights replicated, no
collectives. All GEMMs run in bf16 with fp32 PSUM accumulation (simulated
end-to-end rel err ~5e-4); softmax statistics, normalization and the
residual path stay fp32.

Layout strategy: everything stays "transposed" ([feature, token]) so the
whole chain — qk projection, scores, AV, out projection — needs zero
on-chip transposes:
  qkT[3C', N]  = W_proj_slices.T @ x_r          (lhsT = W_proj, rhs = x natural)
  attT[j, i]   = k @ q.T                        (lhsT = kT cols, rhs = qT)
  E            = exp(attT * scale)              (ScalarE, PSUM -> SBUF, bf16)
  outT[d, i]   = v.T @ E  (lhsT = v natural)    + ones-lhsT matmul -> denominator
  resT[c, i]   = W_out.T @ concatT + bias + x_r (exact output DRAM layout)
The softmax denominator comes from a [128,128] ones lhsT matmul over E's
j-tiles: every PSUM partition row holds s[i], i.e. already broadcast.

Scheduling notes (measured on HW):
 - DMAs ordered so the first head's weights + x land first; dummy bf16
   warmup matmuls bridge the initial DMA wait and keep the PE clock-gate
   (HAM) warm so real matmuls start at 2.4 GHz.
 - PSUM->SBUF copies ride the ScalarEngine; the DVE is kept nearly
   dedicated to the softmax drain (reciprocal_approx_fast + normalize
   muls) so AV accumulator banks recycle fast.
 - Weights/x are DMA'd as fp32 and cast to bf16 on-chip (DMA cannot
   convert dtypes).
"""
import sys

try:
    import concourse.bass as bass  # noqa: F401
except ImportError:
    sys.path.insert(0, "/opt/trn_rl_repo")

from contextlib import ExitStack

import numpy as np

import concourse.bass as bass
import concourse.mybir as mybir
import concourse.tile as tile
from concourse import bacc
from concourse.bass_utils import run_bass_kernel_spmd

F32 = mybir.dt.float32
BF16 = mybir.dt.bfloat16
FP8 = mybir.dt.float8e5
EXP = mybir.ActivationFunctionType.Exp
IDENT = mybir.ActivationFunctionType.Identity

B_PER_CORE = 2   # 16 images / 8 cores
C = 256          # channels == head dim
N = 1024         # tokens (32*32)
HEADS = 4
SCALE = C ** -0.5
N_CORES = 8


def _build():
    nc = bacc.Bacc("TRN2", debug=False, num_devices=N_CORES)
    x_d = nc.declare_dram_parameter("x", [B_PER_CORE, C, N], F32, isOutput=False)
    wp_d = nc.declare_dram_parameter("W_proj", [C, 3 * HEADS * C], F32, isOutput=False)
    bp_d = nc.declare_dram_parameter("b_proj", [3 * HEADS * C], F32, isOutput=False)
    wo_d = nc.declare_dram_parameter("W_out", [HEADS * C, C], F32, isOutput=False)
    bo_d = nc.declare_dram_parameter("b_out", [C], F32, isOutput=False)
    out_d = nc.declare_dram_parameter("out", [B_PER_CORE, C, N], F32, isOutput=True)

    with tile.TileContext(nc) as tc, ExitStack() as ctx:
        pool = ctx.enter_context(tc.tile_pool(name="persist", bufs=1))
        stage_pool = ctx.enter_context(tc.tile_pool(name="stage", bufs=3))
        xr_pool = ctx.enter_context(tc.tile_pool(name="xr", bufs=2))
        xb_pool = ctx.enter_context(tc.tile_pool(name="xb", bufs=2))
        v2_pool = ctx.enter_context(tc.tile_pool(name="v2", bufs=1))
        qk_pool = ctx.enter_context(tc.tile_pool(name="qk", bufs=2))
        e_pool = ctx.enter_context(tc.tile_pool(name="e", bufs=2))
        e8_pool = ctx.enter_context(tc.tile_pool(name="e8", bufs=2))
        cat_pool = ctx.enter_context(tc.tile_pool(name="cat", bufs=1))
        r_pool = ctx.enter_context(tc.tile_pool(name="r", bufs=2))
        xrb_pool = ctx.enter_context(tc.tile_pool(name="xrb", bufs=2))
        out_pool = ctx.enter_context(tc.tile_pool(name="outs", bufs=4))
        ps_work = ctx.enter_context(tc.tile_pool(name="psw", bufs=5, space="PSUM"))
        ps_acc = ctx.enter_context(tc.tile_pool(name="psa", bufs=2, space="PSUM"))
        ps_s = ctx.enter_context(tc.tile_pool(name="pss", bufs=1, space="PSUM"))

        # ---- DMAs + on-chip bf16 casts, first-needed data first ----
        xr_tiles = []
        xr = xr_pool.tile([128, 2, N], F32, tag="xr")
        for kt in range(2):
            for isl in range(2):
                nc.sync.dma_start(
                    out=xr[:, kt, isl * 512:(isl + 1) * 512],
                    in_=x_d[0, kt * 128:(kt + 1) * 128, isl * 512:(isl + 1) * 512])
        xr_tiles.append(xr)

        w_sb = pool.tile([128, 2, 3072], BF16)  # W_proj k-tiles, per-head chunks
        b_sb = None
        for h in range(HEADS):
            for kt in range(2):
                ws = stage_pool.tile([128, 768], F32, tag="wstage")
                nc.sync.dma_start(
                    out=ws[:],
                    in_=wp_d[kt * 128:(kt + 1) * 128, h * 768:(h + 1) * 768])
                nc.vector.tensor_copy(w_sb[:, kt, h * 768:(h + 1) * 768], ws[:])
            if h == 0:
                # biases: needed by the first qk PSUM->SBUF copy, not the MMs
                b_sb = pool.tile([128, 24], F32)  # b_proj, tile t
                nc.sync.dma_start(
                    out=b_sb[:], in_=bp_d[:].rearrange("(t p) -> p t", p=128))
                bo_sb = pool.tile([128, 2], F32)
                nc.sync.dma_start(out=bo_sb[:],
                                  in_=bo_d[:].rearrange("(t p) -> p t", p=128))

        # second image's x: queued last, prefetched during image-0 compute
        xr = xr_pool.tile([128, 2, N], F32, tag="xr")
        for kt in range(2):
            nc.sync.dma_start(out=xr[:, kt, :],
                              in_=x_d[1, kt * 128:(kt + 1) * 128, :])
        xr_tiles.append(xr)

        # ---- small constants ----
        ones_f = pool.tile([128, 512], F32)
        nc.vector.memset(ones_f[:], 1.0)
        ones_w = pool.tile([128, 512], BF16)
        nc.vector.tensor_copy(ones_w[:], ones_f[:])
        ones_sb = ones_w[:, 0:128]
        ones8 = pool.tile([128, 2, 128], FP8)
        nc.vector.tensor_copy(ones8[:],
                              ones_f[:, 0:256].rearrange("p (a b) -> p a b", b=128))

        # dummy matmuls: fill the initial DMA wait + warm the HAM clock gate
        for wi in range(20):
            warm_ps = ps_work.tile([128, 512], F32, tag="work")
            nc.tensor.matmul(out=warm_ps[:], lhsT=ones_sb, rhs=ones_w[:],
                             start=True, stop=True)

        total_bias = pool.tile([128, 2], F32)
        wo_sb = pool.tile([128, 8, 256], BF16)  # W_out k-tiles (loaded mid-image-0)
        zb = pool.tile([128, 8, 2], BF16)

        def qk_proj(xb, h):
            """q,k for head h -> [128, 4(q0 q1 k0 k1), N] bf16."""
            qk = qk_pool.tile([128, 4, N], BF16, tag="qk")
            for mt in range(4):
                cols = h * 768 + mt * 128
                ps0 = ps_work.tile([128, 512], F32, tag="work")
                ps1 = ps_work.tile([128, 512], F32, tag="work")
                ps = [ps0, ps1]
                for kt in range(2):
                    for isl in range(2):
                        nc.tensor.matmul(
                            out=ps[isl][:],
                            lhsT=w_sb[:, kt, cols:cols + 128],
                            rhs=xb[:, kt, isl * 512:(isl + 1) * 512],
                            start=(kt == 0), stop=(kt == 1))
                for isl in range(2):
                    nc.scalar.activation(qk[:, mt, isl * 512:(isl + 1) * 512],
                                         ps[isl][:], IDENT,
                                         bias=b_sb[:, h * 6 + mt:h * 6 + mt + 1])
            return qk

        def v_proj(xb, v2, hp):
            """v for heads 2hp, 2hp+1 -> v2[:, it, h*256+d] (natural layout)."""
            for it in range(8):
                ps = ps_work.tile([128, 512], F32, tag="work")
                for kt in range(2):
                    rhs = w_sb[:, kt, :].rearrange(
                        "p (h c) -> p h c", h=HEADS
                    )[:, 2 * hp:2 * hp + 2, 512:768]
                    nc.tensor.matmul(out=ps[:],
                                     lhsT=xb[:, kt, it * 128:(it + 1) * 128],
                                     rhs=rhs, start=(kt == 0), stop=(kt == 1))
                nc.scalar.copy(v2[:, it, hp * 512:(hp + 1) * 512], ps[:])

        def attT_e(qk):
            """scores attT[j, i] -> E = exp(attT * scale) (+ fp8 shadow for s)."""
            e_t = e_pool.tile([128, 8, N], BF16, tag="e")
            e8 = e8_pool.tile([128, 2, 8, 512], FP8, tag="e8")
            for isl in range(2):
                for jt in range(8):
                    ps = ps_work.tile([128, 512], F32, tag="work")
                    for dt in range(2):
                        nc.tensor.matmul(
                            out=ps[:],
                            lhsT=qk[:, 2 + dt, jt * 128:(jt + 1) * 128],
                            rhs=qk[:, dt, isl * 512:(isl + 1) * 512],
                            start=(dt == 0), stop=(dt == 1))
                    nc.scalar.activation(e_t[:, jt, isl * 512:(isl + 1) * 512],
                                         ps[:], EXP, scale=SCALE)
                    nc.vector.tensor_scalar_mul(
                        e8[:, isl, jt, :],
                        e_t[:, jt, isl * 512:(isl + 1) * 512], 0.0625)
            return e_t, e8

        def av_isl(e_t, e8, v2, cat, h, isl):
            """AV + denominator for one i-half; normalized into concatT.
            The denominator sums fp8 E at DoubleRow half-rate (4 matmuls
            contract 256 j each: j = 256a + p + 128*pair)."""
            o_ps0 = ps_acc.tile([128, 512], F32, tag="acc")
            o_ps1 = ps_acc.tile([128, 512], F32, tag="acc")
            s_ps = ps_s.tile([128, 512], F32, tag="sacc")
            for jt in range(8):
                e_ap = e_t[:, jt, isl * 512:(isl + 1) * 512]
                st, sp = (jt == 0), (jt == 7)
                nc.tensor.matmul(out=o_ps0[:], rhs=e_ap, start=st, stop=sp,
                                 lhsT=v2[:, jt, h * 256:h * 256 + 128])
                nc.tensor.matmul(out=o_ps1[:], rhs=e_ap, start=st, stop=sp,
                                 lhsT=v2[:, jt, h * 256 + 128:h * 256 + 256])
            for a in range(4):
                nc.tensor.matmul(
                    out=s_ps[:], lhsT=ones8[:],
                    rhs=e8[:, isl, 2 * a:2 * a + 2, :],
                    perf_mode=mybir.MatmulPerfMode.DoubleRow,
                    start=(a == 0), stop=(a == 3))
            r_sb = r_pool.tile([128, 512], F32, tag="r")
            nc.vector.reciprocal_approx_fast(r_sb[:], s_ps[:])
            MUL = mybir.AluOpType.mult
            nc.vector.scalar_tensor_tensor(
                cat[:, 2 * h, isl * 512:(isl + 1) * 512], o_ps0[:], 0.0625,
                r_sb[:], MUL, MUL)
            nc.vector.scalar_tensor_tensor(
                cat[:, 2 * h + 1, isl * 512:(isl + 1) * 512], o_ps1[:], 0.0625,
                r_sb[:], MUL, MUL)

        for b in range(B_PER_CORE):
            xr = xr_tiles[b]
            xb = xb_pool.tile([128, 2, N], BF16, tag="xb")
            nc.scalar.copy(xb[:], xr[:])
            v2 = v2_pool.tile([128, 8, 1024], BF16, tag="v2")
            cat = cat_pool.tile([128, 8, N], BF16, tag="cat")

            qk = qk_proj(xb, 0)
            v_proj(xb, v2, 0)
            e_t, e8 = attT_e(qk)
            av_isl(e_t, e8, v2, cat, 0, 0)
            av_isl(e_t, e8, v2, cat, 0, 1)
            qk = qk_proj(xb, 1)
            e_t, e8 = attT_e(qk)
            av_isl(e_t, e8, v2, cat, 1, 0)
            av_isl(e_t, e8, v2, cat, 1, 1)
            if b == 0:
                for kt in range(8):
                    ws = stage_pool.tile([128, 256], F32, tag="wostage")
                    nc.sync.dma_start(out=ws[:],
                                      in_=wo_d[kt * 128:(kt + 1) * 128, :])
                    nc.vector.tensor_copy(wo_sb[:, kt, :], ws[:])
                zscr = stage_pool.tile([128, 16], F32, tag="zscr")
                nc.vector.memset(zscr[:], 0.0)
                nc.vector.tensor_copy(zb[:],
                                      zscr[:].rearrange("p (a b) -> p a b", b=2))
                for kt in range(8):
                    hh, dt = kt // 2, kt % 2
                    nc.vector.tensor_copy(
                        zb[:, kt, 0:1],
                        b_sb[:, hh * 6 + 4 + dt:hh * 6 + 5 + dt])

            qk = qk_proj(xb, 2)
            v_proj(xb, v2, 1)
            e_t, e8 = attT_e(qk)
            av_isl(e_t, e8, v2, cat, 2, 0)
            av_isl(e_t, e8, v2, cat, 2, 1)
            qk = qk_proj(xb, 3)
            e_t, e8 = attT_e(qk)
            av_isl(e_t, e8, v2, cat, 3, 0)
            av_isl(e_t, e8, v2, cat, 3, 1)

            if b == 0:
                # b_v folds through softmax (weights sum to 1) and W_out:
                # total_bias[c] = b_out[c] + sum_hd b_v[hd] * W_out[hd, c].
                # Deferred here so it doesn't stall the PE on the W_out DMA.
                for ct in range(2):
                    bias_ps = ps_work.tile([128, 2], F32, tag="work")
                    for kt in range(8):
                        nc.tensor.matmul(out=bias_ps[:],
                                         lhsT=wo_sb[:, kt, ct * 128:(ct + 1) * 128],
                                         rhs=zb[:, kt, :],
                                         start=(kt == 0), stop=(kt == 7))
                    nc.vector.tensor_add(total_bias[:, ct:ct + 1], bias_ps[:, 0:1],
                                         bo_sb[:, ct:ct + 1])

            # residual + bias, broadcast along tokens: xrb = x_r + total_bias
            xrb = xrb_pool.tile([128, 2, N], F32, tag="xrb")
            for ct in range(2):
                nc.scalar.activation(xrb[:, ct, :], xr[:, ct, :],
                                     IDENT, bias=total_bias[:, ct:ct + 1])

            # ---- out projection + residual, already in output layout ----
            for ct in range(2):
                for isl in range(2):
                    res_ps = ps_work.tile([128, 512], F32, tag="work")
                    for kt in range(8):
                        nc.tensor.matmul(
                            out=res_ps[:],
                            lhsT=wo_sb[:, kt, ct * 128:(ct + 1) * 128],
                            rhs=cat[:, kt, isl * 512:(isl + 1) * 512],
                            start=(kt == 0), stop=(kt == 7))
                    o_sb = out_pool.tile([128, 512], F32, tag="o_sb")
                    nc.vector.tensor_add(o_sb[:], res_ps[:],
                                         xrb[:, ct, isl * 512:(isl + 1) * 512])
                    nc.sync.dma_start(
                        out=out_d[b, ct * 128:(ct + 1) * 128,
                                  isl * 512:(isl + 1) * 512],
                        in_=o_sb[:])

    nc.compile()
    return nc


_NC = None


def kernel(x, W_proj, b_proj, W_out, b_out):
    global _NC
    if _NC is None:
        _NC = _build()
    x = np.ascontiguousarray(x, dtype=np.float32).reshape(16, C, N)
    in_maps = [
        {
            "x": x[i * B_PER_CORE:(i + 1) * B_PER_CORE],
            "W_proj": np.ascontiguousarray(W_proj, dtype=np.float32),
            "b_proj": np.ascontiguousarray(b_proj, dtype=np.float32),
            "W_out": np.ascontiguousarray(W_out, dtype=np.float32),
            "b_out": np.ascontiguousarray(b_out, dtype=np.float32),
        }
        for i in range(N_CORES)
    ]
    res = run_bass_kernel_spmd(_NC, in_maps, core_ids=list(range(N_CORES)))
    out = np.concatenate([res.results[i]["out"] for i in range(N_CORES)], axis=0)
    return out.reshape(16, C, 32, 32)



# revision 4
# speedup vs baseline: 1.0412x; 1.0412x over previous
"""Multi-head attention (B=16, C=256, N=1024, H=4 heads) on 8 TRN2 NeuronCores.

Data-parallel over batch: 2 images per core, weights replicated, no
collectives. All five GEMM stages (qkv proj, scores, softmax denominator,
AV, out proj) run in fp8 e4m3 with DoubleRow perf mode -- each matmul
contracts 256 rows (2 fp8 weights/cell) in 512 cycles, ~2x the bf16 rate.
fp32 PSUM accumulation throughout; simulated end-to-end rel err ~8e-3
(tolerance 2e-2). Softmax statistics and the residual stay >= bf16.

Layout strategy: everything stays "transposed" ([feature, token]) so the
whole chain needs zero on-chip transposes:
  qk8[4, N]   = W_proj_slices.T @ x8    (DR: lhsT = w8qk [ci,kt,*], rhs = x8)
  attT[j, i]  = k8 @ q8.T               (DR: lhsT/rhs = qk8 slot pairs)
  E8          = exp(attT*scale - ln32)  (ScalarE, PSUM -> e4m3 SBUF direct)
  o[d, i]     = v8.T @ E8   (DR, 4 chunks of 256 j) ; s = ones8.T @ E8
  res[c, i]   = wo8.T @ cat8 (DR) + eye_bf16 @ x_bf16  (residual folded
                into the same PSUM group; drained on ScalarE with bias)

Engine balance (per core, model): PE ~118us of matmuls; DVE ~89us
(qk/v PSUM drains with per-partition bias, softmax reciprocal +
normalize); ScalarE ~81us (exp over the 2x4M-element attention matrix,
fp8 x casts, final drain). E is scaled by 1/32 inside the exp bias so
e4m3 never saturates; the scale cancels between numerator o and
denominator s.

The identity matrix for the residual matmul rides in as an extra DRAM
input supplied by kernel() (np.eye), cast to bf16 on chip. b_proj's q/k
biases are applied on the qk drains (DVE tensor_scalar add); b_v folds
through softmax (weights sum to 1) into total_bias = b_out + b_v @ W_out
computed with tiny fp8 matmuls, applied at the final ScalarE drain.
"""
import sys

try:
    import concourse.bass as bass  # noqa: F401
except ImportError:
    sys.path.insert(0, "/opt/trn_rl_repo")

import math
from contextlib import ExitStack

import numpy as np

import concourse.bass as bass
import concourse.mybir as mybir
import concourse.tile as tile
from concourse import bacc
from concourse.bass_utils import run_bass_kernel_spmd

F32 = mybir.dt.float32
BF16 = mybir.dt.bfloat16
E4 = mybir.dt.float8e4
EXP = mybir.ActivationFunctionType.Exp
IDENT = mybir.ActivationFunctionType.Identity
DR = mybir.MatmulPerfMode.DoubleRow
MUL = mybir.AluOpType.mult

B_PER_CORE = 2   # 16 images / 8 cores
C = 256          # channels == head dim
N = 1024         # tokens (32*32)
HEADS = 4
SCALE = C ** -0.5
N_CORES = 8
NLOG32 = -math.log(32.0)


def _build():
    nc = bacc.Bacc("TRN2", debug=False, num_devices=N_CORES)
    x_d = nc.declare_dram_parameter("x", [B_PER_CORE, C, N], F32, isOutput=False)
    wp_d = nc.declare_dram_parameter("W_proj", [C, 3 * HEADS * C], F32, isOutput=False)
    bp_d = nc.declare_dram_parameter("b_proj", [3 * HEADS * C], F32, isOutput=False)
    wo_d = nc.declare_dram_parameter("W_out", [HEADS * C, C], F32, isOutput=False)
    bo_d = nc.declare_dram_parameter("b_out", [C], F32, isOutput=False)
    eye_d = nc.declare_dram_parameter("eye", [128, 128], F32, isOutput=False)
    out_d = nc.declare_dram_parameter("out", [B_PER_CORE, C, N], F32, isOutput=True)

    with tile.TileContext(nc) as tc, ExitStack() as ctx:
        pool = ctx.enter_context(tc.tile_pool(name="persist", bufs=1))
        stage_pool = ctx.enter_context(tc.tile_pool(name="stage", bufs=3))
        xr_pool = ctx.enter_context(tc.tile_pool(name="xr", bufs=2))
        x8_pool = ctx.enter_context(tc.tile_pool(name="x8", bufs=2))
        xb_pool = ctx.enter_context(tc.tile_pool(name="xb", bufs=2))
        qk_pool = ctx.enter_context(tc.tile_pool(name="qk", bufs=2))
        e_pool = ctx.enter_context(tc.tile_pool(name="e8", bufs=2))
        v_pool = ctx.enter_context(tc.tile_pool(name="v8", bufs=2))
        cat_pool = ctx.enter_context(tc.tile_pool(name="cat", bufs=2))
        r_pool = ctx.enter_context(tc.tile_pool(name="r", bufs=2))
        out_pool = ctx.enter_context(tc.tile_pool(name="outs", bufs=4))
        psA = ctx.enter_context(tc.tile_pool(name="psA", bufs=2, space="PSUM"))
        psB = ctx.enter_context(tc.tile_pool(name="psB", bufs=1, space="PSUM"))
        psC = ctx.enter_context(tc.tile_pool(name="psC", bufs=2, space="PSUM"))

        # ---- DMAs + on-chip fp8 casts, first-needed data first ----
        xr_tiles = []
        xr = xr_pool.tile([128, 2, N], F32, tag="xr")
        for kt in range(2):
            for isl in range(2):
                nc.sync.dma_start(
                    out=xr[:, kt, isl * 512:(isl + 1) * 512],
                    in_=x_d[0, kt * 128:(kt + 1) * 128, isl * 512:(isl + 1) * 512])
        xr_tiles.append(xr)

        # W_proj, rearranged: w8qk cols = h*512 + {q0,q1,k0,k1}*128,
        # w8v cols = h*256 + d. kt (c-tile) is the DoubleRow pair dim.
        w8qk = pool.tile([128, 2, 2048], E4)
        w8v = pool.tile([128, 2, 1024], E4)
        b_sb = None
        for h in range(HEADS):
            for kt in range(2):
                ws = stage_pool.tile([128, 768], F32, tag="wstage")
                nc.sync.dma_start(
                    out=ws[:],
                    in_=wp_d[kt * 128:(kt + 1) * 128, h * 768:(h + 1) * 768])
                nc.vector.tensor_copy(w8qk[:, kt, h * 512:(h + 1) * 512],
                                      ws[:, 0:512])
                nc.vector.tensor_copy(w8v[:, kt, h * 256:(h + 1) * 256],
                                      ws[:, 512:768])
            if h == 0:
                b_sb = pool.tile([128, 24], F32)  # b_proj, tile t
                nc.sync.dma_start(
                    out=b_sb[:], in_=bp_d[:].rearrange("(t p) -> p t", p=128))
                bo_sb = pool.tile([128, 2], F32)
                nc.sync.dma_start(out=bo_sb[:],
                                  in_=bo_d[:].rearrange("(t p) -> p t", p=128))
                eye_f = stage_pool.tile([128, 128], F32, tag="eyestage")
                nc.sync.dma_start(out=eye_f[:], in_=eye_d[:, :])
                eye_bf = pool.tile([128, 128], BF16)
                nc.vector.tensor_copy(eye_bf[:], eye_f[:])

        # second image's x: queued last, prefetched during image-0 compute
        xr = xr_pool.tile([128, 2, N], F32, tag="xr")
        for kt in range(2):
            nc.sync.dma_start(out=xr[:, kt, :],
                              in_=x_d[1, kt * 128:(kt + 1) * 128, :])
        xr_tiles.append(xr)

        # ---- small constants ----
        ones_f = pool.tile([128, 512], F32)
        nc.vector.memset(ones_f[:], 1.0)
        ones_w = pool.tile([128, 512], BF16)
        nc.vector.tensor_copy(ones_w[:], ones_f[:])
        ones8 = pool.tile([128, 2, 128], E4)
        nc.vector.memset(ones8[:], 1.0)
        ebias = pool.tile([128, 1], F32)  # exp bias: -ln(32)
        nc.vector.memset(ebias[:], NLOG32)

        # dummy matmuls: fill the initial DMA wait + warm the HAM clock gate
        for wi in range(10):
            warm_ps = psA.tile([128, 2, 512], F32, tag="A")
            for half in range(2):
                nc.tensor.matmul(out=warm_ps[:, half, :], lhsT=ones_w[:, 0:128],
                                 rhs=ones_w[:], start=True, stop=True)

        wo8 = pool.tile([128, 8, 256], E4)   # W_out k-tiles (loaded mid-image-0)
        zb = pool.tile([128, 8, 2], E4)      # b_v columns for the bias fold
        total_bias = pool.tile([128, 2], F32)

        def qk_proj(x8, qk8, h):
            """q,k for head h -> qk8[128, slot(q0 q1 k0 k1), isl, 512] e4m3."""
            for mt in range(4):
                ps = psA.tile([128, 2, 512], F32, tag="A")
                for isl in range(2):
                    nc.tensor.matmul(
                        out=ps[:, isl, :],
                        lhsT=w8qk[:, 0:2, h * 512 + mt * 128:h * 512 + (mt + 1) * 128],
                        rhs=x8[:, 0:2, isl * 512:(isl + 1) * 512],
                        perf_mode=DR, start=True, stop=True)
                nc.vector.tensor_scalar_add(qk8[:, mt], ps[:],
                                            b_sb[:, h * 6 + mt:h * 6 + mt + 1])

        def v_proj(x8, v8, hp):
            """v for heads 2hp, 2hp+1 -> v8[:, it, h*256+d] (natural layout)."""
            for t in range(4):
                ps = psA.tile([128, 2, 512], F32, tag="A")
                for i2 in range(2):
                    it = 2 * t + i2
                    nc.tensor.matmul(
                        out=ps[:, i2, :],
                        lhsT=x8[:, 0:2, it * 128:(it + 1) * 128],
                        rhs=w8v[:, 0:2, hp * 512:(hp + 1) * 512],
                        perf_mode=DR, start=True, stop=True)
                nc.vector.tensor_copy(v8[:, 2 * t:2 * t + 2, hp * 512:(hp + 1) * 512],
                                      ps[:])

        def scores_exp(qk8, e8, isl):
            """attT[j, i] for one i-half -> E8 = exp(attT*scale - ln32) e4m3."""
            for g in range(4):
                ps = psA.tile([128, 2, 512], F32, tag="A")
                for j2 in range(2):
                    jt = 2 * g + j2
                    nc.tensor.matmul(
                        out=ps[:, j2, :],
                        lhsT=qk8[:, 2:4, jt // 4, (jt % 4) * 128:(jt % 4 + 1) * 128],
                        rhs=qk8[:, 0:2, isl, :],
                        perf_mode=DR, start=True, stop=True)
                nc.scalar.activation(e8[:, 2 * g:2 * g + 2, isl * 512:(isl + 1) * 512],
                                     ps[:], EXP, scale=SCALE, bias=ebias[:, 0:1])

        def av(e8, v8, cat8, h, isl):
            """AV + denominator for one i-half; normalized into cat8."""
            o_ps = psB.tile([128, 2, 512], F32, tag="B")
            s_ps = psC.tile([128, 512], F32, tag="C")
            for a in range(4):
                e_ap = e8[:, 2 * a:2 * a + 2, isl * 512:(isl + 1) * 512]
                for dh in range(2):
                    nc.tensor.matmul(
                        out=o_ps[:, dh, :],
                        lhsT=v8[:, 2 * a:2 * a + 2,
                                h * 256 + dh * 128:h * 256 + (dh + 1) * 128],
                        rhs=e_ap, perf_mode=DR, start=(a == 0), stop=(a == 3))
            for a in range(4):
                nc.tensor.matmul(
                    out=s_ps[:], lhsT=ones8[:],
                    rhs=e8[:, 2 * a:2 * a + 2, isl * 512:(isl + 1) * 512],
                    perf_mode=DR, start=(a == 0), stop=(a == 3))
            r_sb = r_pool.tile([128, 512], F32, tag="r")
            nc.vector.reciprocal_approx_fast(r_sb[:], s_ps[:])
            for dh in range(2):
                nc.vector.scalar_tensor_tensor(
                    cat8[:, 2 * h + dh, isl * 512:(isl + 1) * 512],
                    o_ps[:, dh, :], 1.0, r_sb[:], MUL, MUL)

        for b in range(B_PER_CORE):
            xr = xr_tiles[b]
            x8 = x8_pool.tile([128, 2, N], E4, tag="x8")
            xb = xb_pool.tile([128, 2, N], BF16, tag="xb")
            nc.scalar.copy(x8[:], xr[:])
            nc.scalar.copy(xb[:], xr[:])
            v8 = v_pool.tile([128, 8, 1024], E4, tag="v8")
            cat8 = cat_pool.tile([128, 8, 1024], E4, tag="cat")

            for h in range(HEADS):
                qk8 = qk_pool.tile([128, 4, 2, 512], E4, tag="qk")
                qk_proj(x8, qk8, h)
                if h == 0:
                    v_proj(x8, v8, 0)
                elif h == 2:
                    v_proj(x8, v8, 1)
                e8 = e_pool.tile([128, 8, 1024], E4, tag="e8")
                scores_exp(qk8, e8, 0)
                scores_exp(qk8, e8, 1)
                av(e8, v8, cat8, h, 0)
                av(e8, v8, cat8, h, 1)

                if b == 0 and h == 1:
                    # W_out + b_v staging, deferred so it doesn't stall startup
                    for kt in range(8):
                        ws2 = stage_pool.tile([128, 256], F32, tag="wostage")
                        nc.sync.dma_start(out=ws2[:],
                                          in_=wo_d[kt * 128:(kt + 1) * 128, :])
                        nc.vector.tensor_copy(wo8[:, kt, :], ws2[:])
                    zscr = stage_pool.tile([128, 16], F32, tag="zscr")
                    nc.vector.memset(zscr[:], 0.0)
                    nc.vector.tensor_copy(zb[:],
                                          zscr[:].rearrange("p (a b) -> p a b", b=2))
                    for kt in range(8):
                        hh, dt = kt // 2, kt % 2
                        nc.vector.tensor_copy(
                            zb[:, kt, 0:1],
                            b_sb[:, hh * 6 + 4 + dt:hh * 6 + 5 + dt])

            if b == 0:
                # b_v folds through softmax (weights sum to 1) and W_out:
                # total_bias[c] = b_out[c] + sum_hd b_v[hd] * W_out[hd, c].
                for ct in range(2):
                    bias_ps = psC.tile([128, 2], F32, tag="C")
                    for kt in range(8):
                        nc.tensor.matmul(out=bias_ps[:],
                                         lhsT=wo8[:, kt, ct * 128:(ct + 1) * 128],
                                         rhs=zb[:, kt, :],
                                         start=(kt == 0), stop=(kt == 7))
                    nc.vector.tensor_add(total_bias[:, ct:ct + 1], bias_ps[:, 0:1],
                                         bo_sb[:, ct:ct + 1])

            # ---- out projection + residual (identity matmul), output layout ----
            for ct in range(2):
                ps = psA.tile([128, 2, 512], F32, tag="A")
                for a in range(4):
                    for isl in range(2):
                        nc.tensor.matmul(
                            out=ps[:, isl, :],
                            lhsT=wo8[:, 2 * a:2 * a + 2, ct * 128:(ct + 1) * 128],
                            rhs=cat8[:, 2 * a:2 * a + 2, isl * 512:(isl + 1) * 512],
                            perf_mode=DR, start=(a == 0), stop=False)
                for isl in range(2):
                    nc.tensor.matmul(out=ps[:, isl, :], lhsT=eye_bf[:],
                                     rhs=xb[:, ct, isl * 512:(isl + 1) * 512],
                                     start=False, stop=True)
                o_sb = out_pool.tile([128, 1024], F32, tag="osb")
                nc.scalar.activation(o_sb[:].rearrange("p (a b) -> p a b", b=512),
                                     ps[:], IDENT, bias=total_bias[:, ct:ct + 1])
                nc.sync.dma_start(out=out_d[b, ct * 128:(ct + 1) * 128, :],
                                  in_=o_sb[:])

    nc.compile()
    return nc


_NC = None
_EYE = np.eye(128, dtype=np.float32)


def make_in_maps(x, W_proj, b_proj, W_out, b_out):
    x = np.ascontiguousarray(x, dtype=np.float32).reshape(16, C, N)
    return [
        {
            "x": x[i * B_PER_CORE:(i + 1) * B_PER_CORE],
            "W_proj": np.ascontiguousarray(W_proj, dtype=np.float32),
            "b_proj": np.ascontiguousarray(b_proj, dtype=np.float32),
            "W_out": np.ascontiguousarray(W_out, dtype=np.float32),
            "b_out": np.ascontiguousarray(b_out, dtype=np.float32),
            "eye": _EYE,
        }
        for i in range(N_CORES)
    ]


def kernel(x, W_proj, b_proj, W_out, b_out):
    global _NC
    if _NC is None:
        _NC = _build()
    in_maps = make_in_maps(x, W_proj, b_proj, W_out, b_out)
    res = run_bass_kernel_spmd(_NC, in_maps, core_ids=list(range(N_CORES)))
    out = np.concatenate([res.results[i]["out"] for i in range(N_CORES)], axis=0)
    return out.reshape(16, C, 32, 32)


# revision 9
# speedup vs baseline: 1.1679x; 1.1217x over previous
"""Multi-head attention (B=16, C=256, N=1024, H=4 heads) on 8 TRN2 NeuronCores.

Data-parallel over batch: 2 images per core, weights replicated, no
collectives. All five GEMM stages (qkv proj, scores, softmax denominator,
AV, out proj) run in fp8 e4m3 with DoubleRow perf mode -- each matmul
contracts 256 rows (2 fp8 weights/cell) in 512 cycles, ~2x the bf16 rate.
fp32 PSUM accumulation throughout; simulated end-to-end rel err ~8e-3
(tolerance 2e-2). Softmax statistics and the residual stay >= bf16.

Layout strategy: everything stays "transposed" ([feature, token]) so the
whole chain needs zero on-chip transposes:
  qk8[4, N]   = W_proj_slices.T @ x8    (DR: lhsT = w8qk [ci,kt,*], rhs = x8)
  attT[j, i]  = k8 @ q8.T               (DR: lhsT/rhs = qk8 slot pairs)
  E8          = exp(attT*scale - ln32)  (ScalarE, PSUM -> e4m3 SBUF direct)
  o[d, i]     = v8.T @ E8   (DR, 4 chunks of 256 j) ; s = ones8.T @ E8
  res[c, i]   = wo8.T @ cat8 (DR) + eye_bf16 @ x_bf16  (residual folded
                into the same PSUM group; drained on ScalarE with bias)

Scheduling: engines execute their instruction streams IN ORDER, so the
emission is a software pipeline. Scores matmuls (whose PSUM tiles are
drained by ScalarE exp at ~580ns vs ~300ns/MM production) are woven with
dependency-ready "filler" matmuls pulled from a FIFO: v-proj, next head's
qk proj, previous (head, i-half)'s AV+denominator chain, out-proj.
Per-phase PSUM pools (scores / proj / AV-o / AV-s) keep ring-allocation
waits from coupling unrelated phases.

Engine budget per core (model): PE ~140us of matmuls; DVE ~94us (qk/v
PSUM drains with per-partition bias, softmax reciprocal + normalize);
ScalarE ~79us (exp over the 2x 4M-element attention matrix + final
drains); GPSIMD ~20us (all SBUF->SBUF fp8/bf16 casts). E is scaled by
1/32 inside the exp bias so e4m3 never saturates; the scale cancels
between numerator o and denominator s.

The identity matrix for the residual matmul rides in as an extra DRAM
input supplied by kernel() (np.eye), cast to bf16 on chip. b_proj's q/k
biases are applied on the qk drains (DVE tensor_scalar add); b_v folds
through softmax (weights sum to 1) into total_bias = b_out + b_v @ W_out
computed with tiny fp8 matmuls, applied at the final ScalarE drain.
"""
import sys

try:
    import concourse.bass as bass  # noqa: F401
except ImportError:
    sys.path.insert(0, "/opt/trn_rl_repo")

import math
from collections import deque
from contextlib import ExitStack

import numpy as np

import concourse.bass as bass
import concourse.mybir as mybir
import concourse.tile as tile
from concourse import bacc
from concourse.bass_utils import run_bass_kernel_spmd

F32 = mybir.dt.float32
BF16 = mybir.dt.bfloat16
E4 = mybir.dt.float8e4
EXP = mybir.ActivationFunctionType.Exp
IDENT = mybir.ActivationFunctionType.Identity
DR = mybir.MatmulPerfMode.DoubleRow
MUL = mybir.AluOpType.mult

B_PER_CORE = 2   # 16 images / 8 cores
C = 256          # channels == head dim
N = 1024         # tokens (32*32)
HEADS = 4
SCALE = C ** -0.5
N_CORES = 8
NLOG32 = -math.log(32.0)


def _build():
    nc = bacc.Bacc("TRN2", debug=False, num_devices=N_CORES)
    x_d = nc.declare_dram_parameter("x", [B_PER_CORE, C, N], F32, isOutput=False)
    wp_d = nc.declare_dram_parameter("W_proj", [C, 3 * HEADS * C], F32, isOutput=False)
    bp_d = nc.declare_dram_parameter("b_proj", [3 * HEADS * C], F32, isOutput=False)
    wo_d = nc.declare_dram_parameter("W_out", [HEADS * C, C], F32, isOutput=False)
    bo_d = nc.declare_dram_parameter("b_out", [C], F32, isOutput=False)
    eye_d = nc.declare_dram_parameter("eye", [128, 128], F32, isOutput=False)
    out_d = nc.declare_dram_parameter("out", [B_PER_CORE, C, N], F32, isOutput=True)

    with tile.TileContext(nc) as tc, ExitStack() as ctx:
        pool = ctx.enter_context(tc.tile_pool(name="persist", bufs=1))
        stage_pool = ctx.enter_context(tc.tile_pool(name="stage", bufs=3))
        xr_pool = ctx.enter_context(tc.tile_pool(name="xr", bufs=2))
        x8_pool = ctx.enter_context(tc.tile_pool(name="x8", bufs=2))
        xb_pool = ctx.enter_context(tc.tile_pool(name="xb", bufs=2))
        qk_pool = ctx.enter_context(tc.tile_pool(name="qk", bufs=3))
        e_pool = ctx.enter_context(tc.tile_pool(name="e8", bufs=3))
        v_pool = ctx.enter_context(tc.tile_pool(name="v8", bufs=2))
        cat_pool = ctx.enter_context(tc.tile_pool(name="cat", bufs=2))
        r_pool = ctx.enter_context(tc.tile_pool(name="r", bufs=2))
        out_pool = ctx.enter_context(tc.tile_pool(name="outs", bufs=4))
        psS = ctx.enter_context(tc.tile_pool(name="psS", bufs=2, space="PSUM"))
        psQ = ctx.enter_context(tc.tile_pool(name="psQ", bufs=2, space="PSUM"))
        psB = ctx.enter_context(tc.tile_pool(name="psB", bufs=2, space="PSUM"))
        psC = ctx.enter_context(tc.tile_pool(name="psC", bufs=2, space="PSUM"))

        # ---- constants first: they gate the PE warmup, not DMA ----
        ones_f = pool.tile([128, 512], F32)
        nc.vector.memset(ones_f[:], 1.0)
        ones_w = pool.tile([128, 512], BF16)
        nc.vector.tensor_copy(ones_w[:], ones_f[:])
        ones8 = pool.tile([128, 2, 128], E4)
        nc.vector.memset(ones8[:], 1.0)
        ebias = pool.tile([128, 1], F32)  # exp bias: -ln(32)
        nc.vector.memset(ebias[:], NLOG32)

        # ---- DMAs + GPSIMD fp8 casts, first-needed data first ----
        xr_tiles = []
        xr = xr_pool.tile([128, 2, N], F32, tag="xr")
        for kt in range(2):
            for isl in range(2):
                nc.sync.dma_start(
                    out=xr[:, kt, isl * 512:(isl + 1) * 512],
                    in_=x_d[0, kt * 128:(kt + 1) * 128, isl * 512:(isl + 1) * 512])
        xr_tiles.append(xr)

        # W_proj, rearranged: w8qk cols = h*512 + {q0,q1,k0,k1}*128,
        # w8v cols = h*256 + d. kt (c-tile) is the DoubleRow pair dim.
        w8qk = pool.tile([128, 2, 2048], E4)
        w8v = pool.tile([128, 2, 1024], E4)
        b_sb = None
        for h in range(HEADS):
            for kt in range(2):
                ws = stage_pool.tile([128, 768], F32, tag="wstage")
                nc.sync.dma_start(
                    out=ws[:],
                    in_=wp_d[kt * 128:(kt + 1) * 128, h * 768:(h + 1) * 768])
                nc.gpsimd.tensor_copy(w8qk[:, kt, h * 512:(h + 1) * 512],
                                      ws[:, 0:512])
                nc.gpsimd.tensor_copy(w8v[:, kt, h * 256:(h + 1) * 256],
                                      ws[:, 512:768])
            if h == 0:
                b_sb = pool.tile([128, 24], F32)  # b_proj, tile t
                nc.sync.dma_start(
                    out=b_sb[:], in_=bp_d[:].rearrange("(t p) -> p t", p=128))
                bo_sb = pool.tile([128, 2], F32)
                nc.sync.dma_start(out=bo_sb[:],
                                  in_=bo_d[:].rearrange("(t p) -> p t", p=128))
                eye_f = stage_pool.tile([128, 128], F32, tag="eyestage")
                nc.sync.dma_start(out=eye_f[:], in_=eye_d[:, :])
                eye_bf = pool.tile([128, 128], BF16)
                nc.gpsimd.tensor_copy(eye_bf[:], eye_f[:])

        # second image's x: queued last, prefetched during image-0 compute
        xr = xr_pool.tile([128, 2, N], F32, tag="xr")
        for kt in range(2):
            nc.sync.dma_start(out=xr[:, kt, :],
                              in_=x_d[1, kt * 128:(kt + 1) * 128, :])
        xr_tiles.append(xr)

        # dummy matmuls: fill the initial DMA wait + warm the HAM clock gate
        for wi in range(10):
            warm_ps = psS.tile([128, 512], F32, tag="S")
            nc.tensor.matmul(out=warm_ps[:], lhsT=ones_w[:, 0:128],
                             rhs=ones_w[:], start=True, stop=True)

        wo8 = pool.tile([128, 8, 256], E4)   # W_out k-tiles (loaded mid-image-0)
        zb = pool.tile([128, 8, 2], E4)      # b_v columns for the bias fold
        total_bias = pool.tile([128, 2], F32)

        # ---------- emission helpers (each closure emits ~one matmul) ----------
        fq = deque()

        def fpop(k):
            for _ in range(k):
                if fq:
                    fq.popleft()()

        def qk_mms(x8, qk8, h):
            """8 closures: q,k for head h -> qk8[128, slot, isl, 512] e4m3.
            Emission order matches scores' consumption order."""
            def one(mt, isl):
                def go():
                    ps = psQ.tile([128, 512], F32, tag="Q", name="ps_qk")
                    nc.tensor.matmul(
                        out=ps[:],
                        lhsT=w8qk[:, 0:2,
                                  h * 512 + mt * 128:h * 512 + (mt + 1) * 128],
                        rhs=x8[:, 0:2, isl * 512:(isl + 1) * 512],
                        perf_mode=DR, start=True, stop=True)
                    nc.vector.tensor_scalar_add(
                        qk8[:, mt, isl], ps[:],
                        b_sb[:, h * 6 + mt:h * 6 + mt + 1])
                return go
            order = [(0, 0), (1, 0), (2, 0), (3, 0), (2, 1), (3, 1), (0, 1), (1, 1)]
            return [one(mt, isl) for mt, isl in order]

        def v_mms(x8, v8):
            """16 closures: v for all heads -> v8[:, it, h*256+d]."""
            def one(it, hp):
                def go():
                    ps = psQ.tile([128, 512], F32, tag="Q", name="ps_v")
                    nc.tensor.matmul(
                        out=ps[:],
                        lhsT=x8[:, 0:2, it * 128:(it + 1) * 128],
                        rhs=w8v[:, 0:2, hp * 512:(hp + 1) * 512],
                        perf_mode=DR, start=True, stop=True)
                    nc.vector.tensor_copy(v8[:, it, hp * 512:(hp + 1) * 512],
                                          ps[:])
                return go
            return [one(it, hp) for it in range(8) for hp in range(2)]

        def av_mms(e8, v8, cat8, h, isl):
            """12 closures: AV + denominator for one i-half -> cat8 (normalized)."""
            o_ps = [None, None]
            s_ps = [None]

            def mm_o(a, dh):
                def go():
                    if o_ps[dh] is None:
                        o_ps[dh] = psB.tile([128, 512], F32, tag="B", name="o_ps")
                    nc.tensor.matmul(
                        out=o_ps[dh][:],
                        lhsT=v8[:, 2 * a:2 * a + 2,
                                h * 256 + dh * 128:h * 256 + (dh + 1) * 128],
                        rhs=e8[:, 2 * a:2 * a + 2, isl * 512:(isl + 1) * 512],
                        perf_mode=DR, start=(a == 0), stop=(a == 3))
                return go

            def mm_s(a):
                def go():
                    if s_ps[0] is None:
                        s_ps[0] = psC.tile([128, 512], F32, tag="C", name="s_ps")
                    nc.tensor.matmul(
                        out=s_ps[0][:], lhsT=ones8[:],
                        rhs=e8[:, 2 * a:2 * a + 2, isl * 512:(isl + 1) * 512],
                        perf_mode=DR, start=(a == 0), stop=(a == 3))
                    if a == 3:
                        r_sb = r_pool.tile([128, 512], F32, tag="r", name="r_sb")
                        nc.vector.reciprocal_approx_fast(r_sb[:], s_ps[0][:])
                        for dh2 in range(2):
                            nc.vector.scalar_tensor_tensor(
                                cat8[:, 2 * h + dh2, isl * 512:(isl + 1) * 512],
                                o_ps[dh2][:], 1.0, r_sb[:], MUL, MUL)
                return go

            out = []
            for a in range(4):
                out += [mm_o(a, 0), mm_o(a, 1), mm_s(a)]
            return out

        def outproj_mms(b, cat8, xb):
            """20 closures + drains + DMA: res[c, i] with residual + bias."""
            o_sb = [None, None]
            ps = {}

            def mm(ct, isl, a):
                def go():
                    if (ct, isl) not in ps:
                        ps[(ct, isl)] = psQ.tile([128, 512], F32, tag="Q", name="ps_op")
                    nc.tensor.matmul(
                        out=ps[(ct, isl)][:],
                        lhsT=wo8[:, 2 * a:2 * a + 2, ct * 128:(ct + 1) * 128],
                        rhs=cat8[:, 2 * a:2 * a + 2, isl * 512:(isl + 1) * 512],
                        perf_mode=DR, start=(a == 0), stop=False)
                return go

            def mm_eye(ct, isl):
                def go():
                    nc.tensor.matmul(out=ps[(ct, isl)][:], lhsT=eye_bf[:],
                                     rhs=xb[:, ct, isl * 512:(isl + 1) * 512],
                                     start=False, stop=True)
                    if o_sb[ct] is None:
                        o_sb[ct] = out_pool.tile([128, 1024], F32, tag="osb", name="o_sb")
                    nc.scalar.activation(
                        o_sb[ct][:, isl * 512:(isl + 1) * 512], ps[(ct, isl)][:],
                        IDENT, bias=total_bias[:, ct:ct + 1])
                    if isl == 1:
                        nc.sync.dma_start(
                            out=out_d[b, ct * 128:(ct + 1) * 128, :],
                            in_=o_sb[ct][:])
                return go

            out = []
            for ct in range(2):
                for a in range(4):
                    for isl in range(2):
                        out.append(mm(ct, isl, a))
                out += [mm_eye(ct, 0), mm_eye(ct, 1)]
            return out

        def fold_mms():
            """16 tiny closures: total_bias = b_out + b_v @ W_out."""
            bias_ps = {}

            def one(ct, kt):
                def go():
                    if ct not in bias_ps:
                        bias_ps[ct] = psC.tile([128, 2], F32, tag="C", name="bias_ps")
                    nc.tensor.matmul(out=bias_ps[ct][:],
                                     lhsT=wo8[:, kt, ct * 128:(ct + 1) * 128],
                                     rhs=zb[:, kt, :],
                                     start=(kt == 0), stop=(kt == 7))
                    if kt == 7:
                        nc.vector.tensor_add(total_bias[:, ct:ct + 1],
                                             bias_ps[ct][:, 0:1],
                                             bo_sb[:, ct:ct + 1])
                return go
            return [one(ct, kt) for ct in range(2) for kt in range(8)]

        # ---------- software-pipelined emission over units (b, h) ----------
        x8s, xbs, v8s, cats = {}, {}, {}, {}
        markers = {}

        def add_marker(key):
            flag = [False]

            def f():
                flag[0] = True
            fq.append(f)
            markers[key] = flag

        def flush_until(key):
            flag = markers.get(key)
            if flag is not None:
                while not flag[0] and fq:
                    fq.popleft()()

        def image_setup(b):
            x8s[b] = x8_pool.tile([128, 2, N], E4, tag="x8", name="x8t")
            xbs[b] = xb_pool.tile([128, 2, N], BF16, tag="xb", name="xbt")
            nc.gpsimd.tensor_copy(x8s[b][:], xr_tiles[b][:])
            nc.gpsimd.tensor_copy(xbs[b][:], xr_tiles[b][:])

        image_setup(0)
        qk8s = {}
        qk8s[(0, 0)] = qk_pool.tile([128, 4, 2, 512], E4, tag="qk", name="qk8t")
        for f in qk_mms(x8s[0], qk8s[(0, 0)], 0):
            f()  # prologue: nothing to weave with yet

        units = [(b, h) for b in range(B_PER_CORE) for h in range(HEADS)]
        for b, h in units:
            if h == 0:
                v8s[b] = v_pool.tile([128, 8, 1024], E4, tag="v8", name="v8t")
                cats[b] = cat_pool.tile([128, 8, 1024], E4, tag="cat", name="cat8t")
                fq.extend(v_mms(x8s[b], v8s[b]))
            if h < 3:
                qk8s[(b, h + 1)] = qk_pool.tile([128, 4, 2, 512], E4, tag="qk", name="qk8t")
                fq.extend(qk_mms(x8s[b], qk8s[(b, h + 1)], h + 1))
                add_marker((b, h + 1))
            elif b == 0:
                image_setup(1)
                qk8s[(1, 0)] = qk_pool.tile([128, 4, 2, 512], E4, tag="qk", name="qk8t")
                fq.extend(qk_mms(x8s[1], qk8s[(1, 0)], 0))
                add_marker((1, 0))

            if b == 0 and h == 1:
                # W_out + b_v staging on GPSIMD, well before the bias fold
                for kt in range(8):
                    ws2 = stage_pool.tile([128, 256], F32, tag="wostage")
                    nc.sync.dma_start(out=ws2[:],
                                      in_=wo_d[kt * 128:(kt + 1) * 128, :])
                    nc.gpsimd.tensor_copy(wo8[:, kt, :], ws2[:])
                zscr = stage_pool.tile([128, 16], F32, tag="zscr")
                nc.vector.memset(zscr[:], 0.0)
                nc.gpsimd.tensor_copy(zb[:],
                                      zscr[:].rearrange("p (a b) -> p a b", b=2))
                for kt in range(8):
                    hh, dt = kt // 2, kt % 2
                    nc.gpsimd.tensor_copy(
                        zb[:, kt, 0:1],
                        b_sb[:, hh * 6 + 4 + dt:hh * 6 + 5 + dt])
            if b == 1 and h == 0:
                fq.extend(fold_mms())
            if b == 1 and h == 1:
                fq.extend(outproj_mms(0, cats[0], xbs[0]))

            flush_until((b, h))  # qk8(b,h) drains must be emitted before scores
            qk8 = qk8s[(b, h)]
            e8 = e_pool.tile([128, 8, 1024], E4, tag="e8")
            for isl in range(2):
                for jt in range(8):
                    ps = psS.tile([128, 512], F32, tag="S")
                    nc.tensor.matmul(
                        out=ps[:],
                        lhsT=qk8[:, 2:4, jt // 4, (jt % 4) * 128:(jt % 4 + 1) * 128],
                        rhs=qk8[:, 0:2, isl, :],
                        perf_mode=DR, start=True, stop=True)
                    nc.scalar.activation(e8[:, jt, isl * 512:(isl + 1) * 512],
                                         ps[:], EXP, scale=SCALE,
                                         bias=ebias[:, 0:1])
                    fpop(2)
                # AV of this (h, i-half) becomes filler for what follows
                fq.extend(av_mms(e8, v8s[b], cats[b], h, isl))

        # tail: remaining AV of (b1, h3), then out projection of image 1
        fpop(len(fq))
        for f in outproj_mms(1, cats[1], xbs[1]):
            f()

    nc.compile()
    return nc


_NC = None
_EYE = np.eye(128, dtype=np.float32)


def make_in_maps(x, W_proj, b_proj, W_out, b_out):
    x = np.ascontiguousarray(x, dtype=np.float32).reshape(16, C, N)
    return [
        {
            "x": x[i * B_PER_CORE:(i + 1) * B_PER_CORE],
            "W_proj": np.ascontiguousarray(W_proj, dtype=np.float32),
            "b_proj": np.ascontiguousarray(b_proj, dtype=np.float32),
            "W_out": np.ascontiguousarray(W_out, dtype=np.float32),
            "b_out": np.ascontiguousarray(b_out, dtype=np.float32),
            "eye": _EYE,
        }
        for i in range(N_CORES)
    ]


def kernel(x, W_proj, b_proj, W_out, b_out):
    global _NC
    if _NC is None:
        _NC = _build()
    in_maps = make_in_maps(x, W_proj, b_proj, W_out, b_out)
    res = run_bass_kernel_spmd(_NC, in_maps, core_ids=list(range(N_CORES)))
    out = np.concatenate([res.results[i]["out"] for i in range(N_CORES)], axis=0)
    return out.reshape(16, C, 32, 32)


# revision 10
# speedup vs baseline: 1.3564x; 1.1614x over previous
"""Multi-head attention (B=16, C=256, N=1024, H=4 heads) on 8 TRN2 NeuronCores.

Data-parallel over batch: 2 images per core, weights replicated, no
collectives. All five GEMM stages (qkv proj, scores, softmax denominator,
AV, out proj) run in fp8 e4m3 with DoubleRow perf mode -- each matmul
contracts 256 rows (2 fp8 weights/cell) in 512 cycles, ~2x the bf16 rate.
fp32 PSUM accumulation throughout; simulated end-to-end rel err ~8e-3
(tolerance 2e-2). Softmax statistics and the residual stay >= bf16.

Layout strategy: everything stays "transposed" ([feature, token]) so the
whole chain needs zero on-chip transposes:
  qk8[4, N]   = W_proj_slices.T @ x8    (DR: lhsT = w8qk [ci,kt,*], rhs = x8)
  attT[j, i]  = k8 @ q8.T               (DR: lhsT/rhs = qk8 slot pairs)
  E8          = exp(attT*scale - ln32)  (ScalarE, PSUM -> e4m3 SBUF direct)
  o[d, i]     = v8.T @ E8   (DR, 4 chunks of 256 j) ; s = ones8.T @ E8
  res[c, i]   = wo8.T @ cat8 (DR) + eye_bf16 @ x_bf16  (residual folded
                into the same PSUM group; drained on ScalarE with bias)

Scheduling: engines execute their instruction streams IN ORDER, so the
emission is a software pipeline. Scores matmuls (whose PSUM tiles are
drained by ScalarE exp at ~580ns vs ~300ns/MM production) are woven with
dependency-ready "filler" matmuls pulled from a FIFO: v-proj, next head's
qk proj, previous (head, i-half)'s AV+denominator chain, out-proj.
Per-phase PSUM pools (scores / proj / AV-o / AV-s) keep ring-allocation
waits from coupling unrelated phases.

Engine budget per core (model): PE ~140us of matmuls; DVE ~94us (qk/v
PSUM drains with per-partition bias, softmax reciprocal + normalize);
ScalarE ~79us (exp over the 2x 4M-element attention matrix + final
drains); GPSIMD ~20us (all SBUF->SBUF fp8/bf16 casts). E is scaled by
1/32 inside the exp bias so e4m3 never saturates; the scale cancels
between numerator o and denominator s.

The identity matrix for the residual matmul rides in as an extra DRAM
input supplied by kernel() (np.eye), cast to bf16 on chip. b_proj's q/k
biases are applied on the qk drains (DVE tensor_scalar add); b_v folds
through softmax (weights sum to 1) into total_bias = b_out + b_v @ W_out
computed with tiny fp8 matmuls, applied at the final ScalarE drain.
"""
import sys

try:
    import concourse.bass as bass  # noqa: F401
except ImportError:
    sys.path.insert(0, "/opt/trn_rl_repo")

import math
from collections import deque
from contextlib import ExitStack

import numpy as np

import concourse.bass as bass
import concourse.mybir as mybir
import concourse.tile as tile
from concourse import bacc
from concourse.bass_utils import run_bass_kernel_spmd

F32 = mybir.dt.float32
BF16 = mybir.dt.bfloat16
E4 = mybir.dt.float8e4
EXP = mybir.ActivationFunctionType.Exp
IDENT = mybir.ActivationFunctionType.Identity
DR = mybir.MatmulPerfMode.DoubleRow
MUL = mybir.AluOpType.mult

B_PER_CORE = 2   # 16 images / 8 cores
C = 256          # channels == head dim
N = 1024         # tokens (32*32)
HEADS = 4
SCALE = C ** -0.5
N_CORES = 8
NLOG32 = -math.log(32.0)


def _build():
    nc = bacc.Bacc("TRN2", debug=False, num_devices=N_CORES)
    x_d = nc.declare_dram_parameter("x", [B_PER_CORE, C, N], F32, isOutput=False)
    wp_d = nc.declare_dram_parameter("W_proj", [C, 3 * HEADS * C], F32, isOutput=False)
    bp_d = nc.declare_dram_parameter("b_proj", [3 * HEADS * C], F32, isOutput=False)
    wo_d = nc.declare_dram_parameter("W_out", [HEADS * C, C], F32, isOutput=False)
    bo_d = nc.declare_dram_parameter("b_out", [C], F32, isOutput=False)
    eye_d = nc.declare_dram_parameter("eye", [128, 128], F32, isOutput=False)
    out_d = nc.declare_dram_parameter("out", [B_PER_CORE, C, N], F32, isOutput=True)

    with tile.TileContext(nc) as tc, ExitStack() as ctx:
        pool = ctx.enter_context(tc.tile_pool(name="persist", bufs=1))
        stage_pool = ctx.enter_context(tc.tile_pool(name="stage", bufs=3))
        xr_pool = ctx.enter_context(tc.tile_pool(name="xr", bufs=2))
        x8_pool = ctx.enter_context(tc.tile_pool(name="x8", bufs=2))
        xb_pool = ctx.enter_context(tc.tile_pool(name="xb", bufs=2))
        qk_pool = ctx.enter_context(tc.tile_pool(name="qk", bufs=3))
        e_pool = ctx.enter_context(tc.tile_pool(name="e8", bufs=3))
        v_pool = ctx.enter_context(tc.tile_pool(name="v8", bufs=2))
        cat_pool = ctx.enter_context(tc.tile_pool(name="cat", bufs=2))
        r_pool = ctx.enter_context(tc.tile_pool(name="r", bufs=2))
        out_pool = ctx.enter_context(tc.tile_pool(name="outs", bufs=4))
        psS = ctx.enter_context(tc.tile_pool(name="psS", bufs=2, space="PSUM"))
        psQ = ctx.enter_context(tc.tile_pool(name="psQ", bufs=2, space="PSUM"))
        psB = ctx.enter_context(tc.tile_pool(name="psB", bufs=2, space="PSUM"))
        psC = ctx.enter_context(tc.tile_pool(name="psC", bufs=2, space="PSUM"))

        # ---- constants first (GPSIMD memsets): they gate the PE warmup ----
        ones_w = pool.tile([128, 512], BF16)
        nc.gpsimd.memset(ones_w[:], 1.0)
        ones8 = pool.tile([128, 2, 128], E4)
        nc.gpsimd.memset(ones8[:], 1.0)
        ebias = pool.tile([128, 1], F32)  # exp bias: -ln(32)
        nc.gpsimd.memset(ebias[:], NLOG32)

        # ---- DMAs + GPSIMD fp8 casts, first-needed data first ----
        xr_tiles = []
        xr = xr_pool.tile([128, 2, N], F32, tag="xr")
        for kt in range(2):
            for isl in range(2):
                nc.sync.dma_start(
                    out=xr[:, kt, isl * 512:(isl + 1) * 512],
                    in_=x_d[0, kt * 128:(kt + 1) * 128, isl * 512:(isl + 1) * 512])
        xr_tiles.append(xr)

        # W_proj, rearranged: w8qk cols = h*512 + {q0,q1,k0,k1}*128,
        # w8v cols = h*256 + d. kt (c-tile) is the DoubleRow pair dim.
        w8qk = pool.tile([128, 2, 2048], E4)
        w8v = pool.tile([128, 2, 1024], E4)
        b_sb = None
        for h in range(HEADS):
            for kt in range(2):
                ws = stage_pool.tile([128, 768], F32, tag="wstage")
                nc.sync.dma_start(
                    out=ws[:],
                    in_=wp_d[kt * 128:(kt + 1) * 128, h * 768:(h + 1) * 768])
                nc.vector.tensor_copy(w8qk[:, kt, h * 512:(h + 1) * 512],
                                      ws[:, 0:512])
                nc.vector.tensor_copy(w8v[:, kt, h * 256:(h + 1) * 256],
                                      ws[:, 512:768])
            if h == 0:
                b_sb = pool.tile([128, 24], F32)  # b_proj, tile t
                nc.sync.dma_start(
                    out=b_sb[:], in_=bp_d[:].rearrange("(t p) -> p t", p=128))
                bo_sb = pool.tile([128, 2], F32)
                nc.sync.dma_start(out=bo_sb[:],
                                  in_=bo_d[:].rearrange("(t p) -> p t", p=128))
                eye_f = stage_pool.tile([128, 128], F32, tag="eyestage")
                nc.sync.dma_start(out=eye_f[:], in_=eye_d[:, :])
                eye_bf = pool.tile([128, 128], BF16)
                nc.vector.tensor_copy(eye_bf[:], eye_f[:])

        # second image's x: queued last, prefetched during image-0 compute
        xr = xr_pool.tile([128, 2, N], F32, tag="xr")
        for kt in range(2):
            nc.sync.dma_start(out=xr[:, kt, :],
                              in_=x_d[1, kt * 128:(kt + 1) * 128, :])
        xr_tiles.append(xr)

        # dummy matmuls: fill the initial DMA wait + warm the HAM clock gate
        for wi in range(10):
            warm_ps = psS.tile([128, 512], F32, tag="S")
            nc.tensor.matmul(out=warm_ps[:], lhsT=ones_w[:, 0:128],
                             rhs=ones_w[:], start=True, stop=True)

        wo8 = pool.tile([128, 8, 256], E4)   # W_out k-tiles (loaded mid-image-0)
        zb = pool.tile([128, 8, 2], E4)      # b_v columns for the bias fold
        total_bias = pool.tile([128, 2], F32)

        # ---------- emission helpers (each closure emits ~one matmul) ----------
        fq = deque()

        def fpop(k):
            for _ in range(k):
                if fq:
                    fq.popleft()()

        def qk_mms(x8, qk8, h):
            """8 closures: q,k for head h -> qk8[128, slot, isl, 512] e4m3.
            Emission order matches scores' consumption order."""
            def one(mt, isl):
                def go():
                    ps = psQ.tile([128, 512], F32, tag="Q", name="ps_qk")
                    nc.tensor.matmul(
                        out=ps[:],
                        lhsT=w8qk[:, 0:2,
                                  h * 512 + mt * 128:h * 512 + (mt + 1) * 128],
                        rhs=x8[:, 0:2, isl * 512:(isl + 1) * 512],
                        perf_mode=DR, start=True, stop=True)
                    nc.vector.tensor_scalar_add(
                        qk8[:, mt, isl], ps[:],
                        b_sb[:, h * 6 + mt:h * 6 + mt + 1])
                return go
            order = [(0, 0), (1, 0), (2, 0), (3, 0), (2, 1), (3, 1), (0, 1), (1, 1)]
            return [one(mt, isl) for mt, isl in order]

        def v_mms(x8, v8):
            """16 closures: v for all heads -> v8[:, it, h*256+d]."""
            def one(it, hp):
                def go():
                    ps = psQ.tile([128, 512], F32, tag="Q", name="ps_v")
                    nc.tensor.matmul(
                        out=ps[:],
                        lhsT=x8[:, 0:2, it * 128:(it + 1) * 128],
                        rhs=w8v[:, 0:2, hp * 512:(hp + 1) * 512],
                        perf_mode=DR, start=True, stop=True)
                    nc.vector.tensor_copy(v8[:, it, hp * 512:(hp + 1) * 512],
                                          ps[:])
                return go
            return [one(it, hp) for it in range(8) for hp in range(2)]

        def av_mms(e8, v8, cat8, h, isl):
            """12 closures: AV + denominator for one i-half -> cat8 (normalized)."""
            o_ps = [None, None]
            s_ps = [None]

            def mm_o(a, dh):
                def go():
                    if o_ps[dh] is None:
                        o_ps[dh] = psB.tile([128, 512], F32, tag="B", name="o_ps")
                    nc.tensor.matmul(
                        out=o_ps[dh][:],
                        lhsT=v8[:, 2 * a:2 * a + 2,
                                h * 256 + dh * 128:h * 256 + (dh + 1) * 128],
                        rhs=e8[:, 2 * a:2 * a + 2, isl * 512:(isl + 1) * 512],
                        perf_mode=DR, start=(a == 0), stop=(a == 3))
                return go

            def mm_s(a):
                def go():
                    if s_ps[0] is None:
                        s_ps[0] = psC.tile([128, 512], F32, tag="C", name="s_ps")
                    nc.tensor.matmul(
                        out=s_ps[0][:], lhsT=ones8[:],
                        rhs=e8[:, 2 * a:2 * a + 2, isl * 512:(isl + 1) * 512],
                        perf_mode=DR, start=(a == 0), stop=(a == 3))
                    if a == 3:
                        r_sb = r_pool.tile([128, 512], F32, tag="r", name="r_sb")
                        nc.vector.reciprocal_approx_fast(r_sb[:], s_ps[0][:])
                        for dh2 in range(2):
                            nc.vector.scalar_tensor_tensor(
                                cat8[:, 2 * h + dh2, isl * 512:(isl + 1) * 512],
                                o_ps[dh2][:], 1.0, r_sb[:], MUL, MUL)
                return go

            out = []
            for a in range(4):
                out += [mm_o(a, 0), mm_o(a, 1), mm_s(a)]
            return out

        def outproj_mms(b, cat8, xb):
            """20 closures + drains + DMA: res[c, i] with residual + bias."""
            o_sb = [None, None]
            ps = {}

            def mm(ct, isl, a):
                def go():
                    if (ct, isl) not in ps:
                        ps[(ct, isl)] = psQ.tile([128, 512], F32, tag="Q", name="ps_op")
                    nc.tensor.matmul(
                        out=ps[(ct, isl)][:],
                        lhsT=wo8[:, 2 * a:2 * a + 2, ct * 128:(ct + 1) * 128],
                        rhs=cat8[:, 2 * a:2 * a + 2, isl * 512:(isl + 1) * 512],
                        perf_mode=DR, start=(a == 0), stop=False)
                return go

            def mm_eye(ct, isl):
                def go():
                    nc.tensor.matmul(out=ps[(ct, isl)][:], lhsT=eye_bf[:],
                                     rhs=xb[:, ct, isl * 512:(isl + 1) * 512],
                                     start=False, stop=True)
                    if o_sb[ct] is None:
                        o_sb[ct] = out_pool.tile([128, 1024], F32, tag="osb", name="o_sb")
                    nc.scalar.activation(
                        o_sb[ct][:, isl * 512:(isl + 1) * 512], ps[(ct, isl)][:],
                        IDENT, bias=total_bias[:, ct:ct + 1])
                    if isl == 1:
                        nc.sync.dma_start(
                            out=out_d[b, ct * 128:(ct + 1) * 128, :],
                            in_=o_sb[ct][:])
                return go

            out = []
            for ct in range(2):
                for a in range(4):
                    for isl in range(2):
                        out.append(mm(ct, isl, a))
                out += [mm_eye(ct, 0), mm_eye(ct, 1)]
            return out

        def fold_mms():
            """16 tiny closures: total_bias = b_out + b_v @ W_out."""
            bias_ps = {}

            def one(ct, kt):
                def go():
                    if ct not in bias_ps:
                        bias_ps[ct] = psC.tile([128, 2], F32, tag="C", name="bias_ps")
                    nc.tensor.matmul(out=bias_ps[ct][:],
                                     lhsT=wo8[:, kt, ct * 128:(ct + 1) * 128],
                                     rhs=zb[:, kt, :],
                                     start=(kt == 0), stop=(kt == 7))
                    if kt == 7:
                        nc.vector.tensor_add(total_bias[:, ct:ct + 1],
                                             bias_ps[ct][:, 0:1],
                                             bo_sb[:, ct:ct + 1])
                return go
            return [one(ct, kt) for ct in range(2) for kt in range(8)]

        # ---------- software-pipelined emission over units (b, h) ----------
        x8s, xbs, v8s, cats = {}, {}, {}, {}
        markers = {}

        def add_marker(key):
            flag = [False]

            def f():
                flag[0] = True
            fq.append(f)
            markers[key] = flag

        def flush_until(key):
            flag = markers.get(key)
            if flag is not None:
                while not flag[0] and fq:
                    fq.popleft()()

        def image_setup(b):
            x8s[b] = x8_pool.tile([128, 2, N], E4, tag="x8", name="x8t")
            xbs[b] = xb_pool.tile([128, 2, N], BF16, tag="xb", name="xbt")
            nc.scalar.copy(x8s[b][:], xr_tiles[b][:])
            nc.gpsimd.tensor_copy(xbs[b][:], xr_tiles[b][:])

        image_setup(0)
        qk8s = {}
        qk8s[(0, 0)] = qk_pool.tile([128, 4, 2, 512], E4, tag="qk", name="qk8t")
        for f in qk_mms(x8s[0], qk8s[(0, 0)], 0):
            f()  # prologue: nothing to weave with yet

        units = [(b, h) for b in range(B_PER_CORE) for h in range(HEADS)]
        for b, h in units:
            if h == 0:
                v8s[b] = v_pool.tile([128, 8, 1024], E4, tag="v8", name="v8t")
                cats[b] = cat_pool.tile([128, 8, 1024], E4, tag="cat", name="cat8t")
                fq.extend(v_mms(x8s[b], v8s[b]))
            if h < 3:
                qk8s[(b, h + 1)] = qk_pool.tile([128, 4, 2, 512], E4, tag="qk", name="qk8t")
                fq.extend(qk_mms(x8s[b], qk8s[(b, h + 1)], h + 1))
                add_marker((b, h + 1))
            if b == 0 and h == 2:
                image_setup(1)
                qk8s[(1, 0)] = qk_pool.tile([128, 4, 2, 512], E4, tag="qk", name="qk8t")
                fq.extend(qk_mms(x8s[1], qk8s[(1, 0)], 0))
                add_marker((1, 0))

            if b == 0 and h == 1:
                # W_out + b_v staging on GPSIMD, well before the bias fold
                for kt in range(8):
                    ws2 = stage_pool.tile([128, 256], F32, tag="wostage")
                    nc.sync.dma_start(out=ws2[:],
                                      in_=wo_d[kt * 128:(kt + 1) * 128, :])
                    nc.gpsimd.tensor_copy(wo8[:, kt, :], ws2[:])
                zscr = stage_pool.tile([128, 16], F32, tag="zscr")
                nc.vector.memset(zscr[:], 0.0)
                nc.gpsimd.tensor_copy(zb[:],
                                      zscr[:].rearrange("p (a b) -> p a b", b=2))
                for kt in range(8):
                    hh, dt = kt // 2, kt % 2
                    nc.gpsimd.tensor_copy(
                        zb[:, kt, 0:1],
                        b_sb[:, hh * 6 + 4 + dt:hh * 6 + 5 + dt])
            if b == 1 and h == 0:
                fq.extend(fold_mms())
            if b == 1 and h == 1:
                fq.extend(outproj_mms(0, cats[0], xbs[0]))

            flush_until((b, h))  # qk8(b,h) drains must be emitted before scores
            qk8 = qk8s[(b, h)]
            e8 = e_pool.tile([128, 8, 1024], E4, tag="e8")
            for isl in range(2):
                for jt in range(8):
                    ps = psS.tile([128, 512], F32, tag="S")
                    nc.tensor.matmul(
                        out=ps[:],
                        lhsT=qk8[:, 2:4, jt // 4, (jt % 4) * 128:(jt % 4 + 1) * 128],
                        rhs=qk8[:, 0:2, isl, :],
                        perf_mode=DR, start=True, stop=True)
                    nc.scalar.activation(e8[:, jt, isl * 512:(isl + 1) * 512],
                                         ps[:], EXP, scale=SCALE,
                                         bias=ebias[:, 0:1])
                    fpop(2)
                # AV of this (h, i-half) becomes filler for what follows
                fq.extend(av_mms(e8, v8s[b], cats[b], h, isl))

        # tail: remaining AV of (b1, h3), then out projection of image 1
        fpop(len(fq))
        for f in outproj_mms(1, cats[1], xbs[1]):
            f()

    nc.compile()
    return nc


_NC = None
_EYE = np.eye(128, dtype=np.float32)


def make_in_maps(x, W_proj, b_proj, W_out, b_out):
    x = np.ascontiguousarray(x, dtype=np.float32).reshape(16, C, N)
    return [
        {
            "x": x[i * B_PER_CORE:(i + 1) * B_PER_CORE],
            "W_proj": np.ascontiguousarray(W_proj, dtype=np.float32),
            "b_proj": np.ascontiguousarray(b_proj, dtype=np.float32),
            "W_out": np.ascontiguousarray(W_out, dtype=np.float32),
            "b_out": np.ascontiguousarray(b_out, dtype=np.float32),
            "eye": _EYE,
        }
        for i in range(N_CORES)
    ]


def kernel(x, W_proj, b_proj, W_out, b_out):
    global _NC
    if _NC is None:
        _NC = _build()
    in_maps = make_in_maps(x, W_proj, b_proj, W_out, b_out)
    res = run_bass_kernel_spmd(_NC, in_maps, core_ids=list(range(N_CORES)))
    out = np.concatenate([res.results[i]["out"] for i in range(N_CORES)], axis=0)
    return out.reshape(16, C, 32, 32)


# revision 11
# speedup vs baseline: 1.3647x; 1.0061x over previous
"""Multi-head attention (B=16, C=256, N=1024, H=4 heads) on 8 TRN2 NeuronCores.

Data-parallel over batch: 2 images per core, weights replicated, no
collectives. All five GEMM stages (qkv proj, scores, softmax denominator,
AV, out proj) run in fp8 e4m3 with DoubleRow perf mode -- each matmul
contracts 256 rows (2 fp8 weights/cell) in 512 cycles, ~2x the bf16 rate.
fp32 PSUM accumulation throughout; simulated end-to-end rel err ~8e-3
(tolerance 2e-2). Softmax statistics and the residual stay >= bf16.

Layout strategy: everything stays "transposed" ([feature, token]) so the
whole chain needs zero on-chip transposes:
  qk8[4, N]   = W_proj_slices.T @ x8    (DR: lhsT = w8qk [ci,kt,*], rhs = x8)
  attT[j, i]  = k8 @ q8.T               (DR: lhsT/rhs = qk8 slot pairs)
  E8          = exp(attT*scale - ln32)  (ScalarE, PSUM -> e4m3 SBUF direct)
  o[d, i]     = v8.T @ E8   (DR, 4 chunks of 256 j) ; s = ones8.T @ E8
  res[c, i]   = wo8.T @ cat8 (DR) + eye_bf16 @ x_bf16  (residual folded
                into the same PSUM group; drained on ScalarE with bias)

Scheduling: engines execute their instruction streams IN ORDER, so the
emission is a software pipeline. Scores matmuls (whose PSUM tiles are
drained by ScalarE exp at ~580ns vs ~300ns/MM production) are woven with
dependency-ready "filler" matmuls pulled from a FIFO: v-proj, next head's
qk proj, previous (head, i-half)'s AV+denominator chain, out-proj.
Per-phase PSUM pools (scores / proj / AV-o / AV-s) keep ring-allocation
waits from coupling unrelated phases.

Engine budget per core (model): PE ~140us of matmuls; DVE ~94us (qk/v
PSUM drains with per-partition bias, softmax reciprocal + normalize);
ScalarE ~79us (exp over the 2x 4M-element attention matrix + final
drains); GPSIMD ~20us (all SBUF->SBUF fp8/bf16 casts). E is scaled by
1/32 inside the exp bias so e4m3 never saturates; the scale cancels
between numerator o and denominator s.

The identity matrix for the residual matmul rides in as an extra DRAM
input supplied by kernel() (np.eye), cast to bf16 on chip. b_proj's q/k
biases are applied on the qk drains (DVE tensor_scalar add); b_v folds
through softmax (weights sum to 1) into total_bias = b_out + b_v @ W_out
computed with tiny fp8 matmuls, applied at the final ScalarE drain.
"""
import sys

try:
    import concourse.bass as bass  # noqa: F401
except ImportError:
    sys.path.insert(0, "/opt/trn_rl_repo")

import math
from collections import deque
from contextlib import ExitStack

import numpy as np

import concourse.bass as bass
import concourse.mybir as mybir
import concourse.tile as tile
from concourse import bacc
from concourse.bass_utils import run_bass_kernel_spmd

F32 = mybir.dt.float32
BF16 = mybir.dt.bfloat16
E4 = mybir.dt.float8e4
EXP = mybir.ActivationFunctionType.Exp
IDENT = mybir.ActivationFunctionType.Identity
DR = mybir.MatmulPerfMode.DoubleRow
MUL = mybir.AluOpType.mult

B_PER_CORE = 2   # 16 images / 8 cores
C = 256          # channels == head dim
N = 1024         # tokens (32*32)
HEADS = 4
SCALE = C ** -0.5
N_CORES = 8
NLOG32 = -math.log(32.0)


def _build():
    nc = bacc.Bacc("TRN2", debug=False, num_devices=N_CORES)
    x_d = nc.declare_dram_parameter("x", [B_PER_CORE, C, N], F32, isOutput=False)
    wp_d = nc.declare_dram_parameter("W_proj", [C, 3 * HEADS * C], F32, isOutput=False)
    bp_d = nc.declare_dram_parameter("b_proj", [3 * HEADS * C], F32, isOutput=False)
    wo_d = nc.declare_dram_parameter("W_out", [HEADS * C, C], F32, isOutput=False)
    bo_d = nc.declare_dram_parameter("b_out", [C], F32, isOutput=False)
    eye_d = nc.declare_dram_parameter("eye", [128, 128], F32, isOutput=False)
    out_d = nc.declare_dram_parameter("out", [B_PER_CORE, C, N], F32, isOutput=True)

    with tile.TileContext(nc) as tc, ExitStack() as ctx:
        pool = ctx.enter_context(tc.tile_pool(name="persist", bufs=1))
        stage_pool = ctx.enter_context(tc.tile_pool(name="stage", bufs=6))
        xr_pool = ctx.enter_context(tc.tile_pool(name="xr", bufs=2))
        x8_pool = ctx.enter_context(tc.tile_pool(name="x8", bufs=2))
        xb_pool = ctx.enter_context(tc.tile_pool(name="xb", bufs=2))
        qk_pool = ctx.enter_context(tc.tile_pool(name="qk", bufs=3))
        e_pool = ctx.enter_context(tc.tile_pool(name="e8", bufs=3))
        v_pool = ctx.enter_context(tc.tile_pool(name="v8", bufs=2))
        cat_pool = ctx.enter_context(tc.tile_pool(name="cat", bufs=2))
        r_pool = ctx.enter_context(tc.tile_pool(name="r", bufs=2))
        out_pool = ctx.enter_context(tc.tile_pool(name="outs", bufs=4))
        psS = ctx.enter_context(tc.tile_pool(name="psS", bufs=2, space="PSUM"))
        psQ = ctx.enter_context(tc.tile_pool(name="psQ", bufs=2, space="PSUM"))
        psB = ctx.enter_context(tc.tile_pool(name="psB", bufs=2, space="PSUM"))
        psC = ctx.enter_context(tc.tile_pool(name="psC", bufs=2, space="PSUM"))

        # ---- constants first (GPSIMD memsets): they gate the PE warmup ----
        ones_w = pool.tile([128, 512], BF16)
        nc.gpsimd.memset(ones_w[:], 1.0)
        ones8 = pool.tile([128, 2, 128], E4)
        nc.gpsimd.memset(ones8[:], 1.0)
        ebias = pool.tile([128, 1], F32)  # exp bias: -ln(32)
        nc.gpsimd.memset(ebias[:], NLOG32)

        # ---- DMAs + GPSIMD fp8 casts, first-needed data first ----
        xr_tiles = []
        xr = xr_pool.tile([128, 2, N], F32, tag="xr")
        for isl in range(2):
            for kt in range(2):
                nc.sync.dma_start(
                    out=xr[:, kt, isl * 512:(isl + 1) * 512],
                    in_=x_d[0, kt * 128:(kt + 1) * 128, isl * 512:(isl + 1) * 512])
        xr_tiles.append(xr)

        # W_proj, rearranged: w8qk cols = h*512 + {q0,q1,k0,k1}*128,
        # w8v cols = h*256 + d. kt (c-tile) is the DoubleRow pair dim.
        w8qk = pool.tile([128, 2, 2048], E4)
        w8v = pool.tile([128, 2, 1024], E4)
        b_sb = None
        deferred_wcasts = []
        for h in range(HEADS):
            for kt in range(2):
                ws = stage_pool.tile([128, 768], F32, tag="wstage")
                nc.sync.dma_start(
                    out=ws[:],
                    in_=wp_d[kt * 128:(kt + 1) * 128, h * 768:(h + 1) * 768])
                if h == 0:
                    nc.vector.tensor_copy(w8qk[:, kt, h * 512:(h + 1) * 512],
                                          ws[:, 0:512])
                    nc.vector.tensor_copy(w8v[:, kt, h * 256:(h + 1) * 256],
                                          ws[:, 512:768])
                else:
                    deferred_wcasts.append((ws, h, kt))
            if h == 0:
                b_sb = pool.tile([128, 24], F32)  # b_proj, tile t
                nc.sync.dma_start(
                    out=b_sb[:], in_=bp_d[:].rearrange("(t p) -> p t", p=128))
                bo_sb = pool.tile([128, 2], F32)
                nc.sync.dma_start(out=bo_sb[:],
                                  in_=bo_d[:].rearrange("(t p) -> p t", p=128))
                eye_f = stage_pool.tile([128, 128], F32, tag="eyestage")
                nc.sync.dma_start(out=eye_f[:], in_=eye_d[:, :])
                eye_bf = pool.tile([128, 128], BF16)
                nc.vector.tensor_copy(eye_bf[:], eye_f[:])

        # second image's x: queued last, prefetched during image-0 compute
        xr = xr_pool.tile([128, 2, N], F32, tag="xr")
        for kt in range(2):
            nc.sync.dma_start(out=xr[:, kt, :],
                              in_=x_d[1, kt * 128:(kt + 1) * 128, :])
        xr_tiles.append(xr)

        # dummy matmuls: fill the initial DMA wait + warm the HAM clock gate
        for wi in range(24):
            warm_ps = psS.tile([128, 512], F32, tag="S")
            nc.tensor.matmul(out=warm_ps[:], lhsT=ones_w[:, 0:128],
                             rhs=ones_w[:], start=True, stop=True)

        wo8 = pool.tile([128, 8, 256], E4)   # W_out k-tiles (loaded mid-image-0)
        zb = pool.tile([128, 8, 2], E4)      # b_v columns for the bias fold
        total_bias = pool.tile([128, 2], F32)

        # ---------- emission helpers (each closure emits ~one matmul) ----------
        fq = deque()

        def fpop(k):
            for _ in range(k):
                if fq:
                    fq.popleft()()

        def qk_mms(x8, qk8, h):
            """8 closures: q,k for head h -> qk8[128, slot, isl, 512] e4m3.
            Emission order matches scores' consumption order."""
            def one(mt, isl):
                def go():
                    ps = psQ.tile([128, 512], F32, tag="Q", name="ps_qk")
                    nc.tensor.matmul(
                        out=ps[:],
                        lhsT=w8qk[:, 0:2,
                                  h * 512 + mt * 128:h * 512 + (mt + 1) * 128],
                        rhs=x8[:, 0:2, isl * 512:(isl + 1) * 512],
                        perf_mode=DR, start=True, stop=True)
                    nc.vector.tensor_scalar_add(
                        qk8[:, mt, isl], ps[:],
                        b_sb[:, h * 6 + mt:h * 6 + mt + 1])
                return go
            order = [(0, 0), (1, 0), (2, 0), (3, 0), (2, 1), (3, 1), (0, 1), (1, 1)]
            return [one(mt, isl) for mt, isl in order]

        def v_mms(x8, v8):
            """16 closures: v for all heads -> v8[:, it, h*256+d]."""
            def one(it, hp):
                def go():
                    ps = psQ.tile([128, 512], F32, tag="Q", name="ps_v")
                    nc.tensor.matmul(
                        out=ps[:],
                        lhsT=x8[:, 0:2, it * 128:(it + 1) * 128],
                        rhs=w8v[:, 0:2, hp * 512:(hp + 1) * 512],
                        perf_mode=DR, start=True, stop=True)
                    nc.vector.tensor_copy(v8[:, it, hp * 512:(hp + 1) * 512],
                                          ps[:])
                return go
            return [one(it, hp) for it in range(8) for hp in range(2)]

        def av_mms(e8, v8, cat8, h, isl):
            """12 closures: AV + denominator for one i-half -> cat8 (normalized)."""
            o_ps = [None, None]
            s_ps = [None]

            def mm_o(a, dh):
                def go():
                    if o_ps[dh] is None:
                        o_ps[dh] = psB.tile([128, 512], F32, tag="B", name="o_ps")
                    nc.tensor.matmul(
                        out=o_ps[dh][:],
                        lhsT=v8[:, 2 * a:2 * a + 2,
                                h * 256 + dh * 128:h * 256 + (dh + 1) * 128],
                        rhs=e8[:, 2 * a:2 * a + 2, isl * 512:(isl + 1) * 512],
                        perf_mode=DR, start=(a == 0), stop=(a == 3))
                return go

            def mm_s(a):
                def go():
                    if s_ps[0] is None:
                        s_ps[0] = psC.tile([128, 512], F32, tag="C", name="s_ps")
                    nc.tensor.matmul(
                        out=s_ps[0][:], lhsT=ones8[:],
                        rhs=e8[:, 2 * a:2 * a + 2, isl * 512:(isl + 1) * 512],
                        perf_mode=DR, start=(a == 0), stop=(a == 3))
                    if a == 3:
                        r_sb = r_pool.tile([128, 512], F32, tag="r", name="r_sb")
                        nc.vector.reciprocal_approx_fast(r_sb[:], s_ps[0][:])
                        for dh2 in range(2):
                            nc.vector.scalar_tensor_tensor(
                                cat8[:, 2 * h + dh2, isl * 512:(isl + 1) * 512],
                                o_ps[dh2][:], 1.0, r_sb[:], MUL, MUL)
                return go

            out = []
            for a in range(4):
                out += [mm_o(a, 0), mm_o(a, 1), mm_s(a)]
            return out

        def outproj_mms(b, cat8, xb):
            """20 closures + drains + DMA: res[c, i] with residual + bias."""
            o_sb = [None, None]
            ps = {}

            def mm(ct, isl, a):
                def go():
                    if (ct, isl) not in ps:
                        ps[(ct, isl)] = psQ.tile([128, 512], F32, tag="Q", name="ps_op")
                    nc.tensor.matmul(
                        out=ps[(ct, isl)][:],
                        lhsT=wo8[:, 2 * a:2 * a + 2, ct * 128:(ct + 1) * 128],
                        rhs=cat8[:, 2 * a:2 * a + 2, isl * 512:(isl + 1) * 512],
                        perf_mode=DR, start=(a == 0), stop=False)
                return go

            def mm_eye(ct, isl):
                def go():
                    nc.tensor.matmul(out=ps[(ct, isl)][:], lhsT=eye_bf[:],
                                     rhs=xb[:, ct, isl * 512:(isl + 1) * 512],
                                     start=False, stop=True)
                    if o_sb[ct] is None:
                        o_sb[ct] = out_pool.tile([128, 1024], F32, tag="osb", name="o_sb")
                    nc.scalar.activation(
                        o_sb[ct][:, isl * 512:(isl + 1) * 512], ps[(ct, isl)][:],
                        IDENT, bias=total_bias[:, ct:ct + 1])
                    if isl == 1:
                        nc.sync.dma_start(
                            out=out_d[b, ct * 128:(ct + 1) * 128, :],
                            in_=o_sb[ct][:])
                return go

            out = []
            for ct in range(2):
                for a in range(4):
                    for isl in range(2):
                        out.append(mm(ct, isl, a))
                out += [mm_eye(ct, 0), mm_eye(ct, 1)]
            return out

        def fold_mms():
            """16 tiny closures: total_bias = b_out + b_v @ W_out."""
            bias_ps = {}

            def one(ct, kt):
                def go():
                    if ct not in bias_ps:
                        bias_ps[ct] = psC.tile([128, 2], F32, tag="C", name="bias_ps")
                    nc.tensor.matmul(out=bias_ps[ct][:],
                                     lhsT=wo8[:, kt, ct * 128:(ct + 1) * 128],
                                     rhs=zb[:, kt, :],
                                     start=(kt == 0), stop=(kt == 7))
                    if kt == 7:
                        nc.vector.tensor_add(total_bias[:, ct:ct + 1],
                                             bias_ps[ct][:, 0:1],
                                             bo_sb[:, ct:ct + 1])
                return go
            return [one(ct, kt) for ct in range(2) for kt in range(8)]

        # ---------- software-pipelined emission over units (b, h) ----------
        x8s, xbs, v8s, cats = {}, {}, {}, {}
        markers = {}

        def add_marker(key):
            flag = [False]

            def f():
                flag[0] = True
            fq.append(f)
            markers[key] = flag

        def flush_until(key):
            flag = markers.get(key)
            if flag is not None:
                while not flag[0] and fq:
                    fq.popleft()()

        def image_setup(b):
            x8s[b] = x8_pool.tile([128, 2, N], E4, tag="x8", name="x8t")
            xbs[b] = xb_pool.tile([128, 2, N], BF16, tag="xb", name="xbt")
            for isl in range(2):
                nc.scalar.copy(x8s[b][:, 0:2, isl * 512:(isl + 1) * 512],
                               xr_tiles[b][:, 0:2, isl * 512:(isl + 1) * 512])
            nc.gpsimd.tensor_copy(xbs[b][:], xr_tiles[b][:])

        image_setup(0)
        qk8s = {}
        qk8s[(0, 0)] = qk_pool.tile([128, 4, 2, 512], E4, tag="qk", name="qk8t")
        for f in qk_mms(x8s[0], qk8s[(0, 0)], 0):
            f()  # prologue: nothing to weave with yet
        for ws, h, kt in deferred_wcasts:
            nc.vector.tensor_copy(w8qk[:, kt, h * 512:(h + 1) * 512],
                                  ws[:, 0:512])
            nc.vector.tensor_copy(w8v[:, kt, h * 256:(h + 1) * 256],
                                  ws[:, 512:768])

        units = [(b, h) for b in range(B_PER_CORE) for h in range(HEADS)]
        for b, h in units:
            if h == 0:
                v8s[b] = v_pool.tile([128, 8, 1024], E4, tag="v8", name="v8t")
                cats[b] = cat_pool.tile([128, 8, 1024], E4, tag="cat", name="cat8t")
                fq.extend(v_mms(x8s[b], v8s[b]))
            nexts = {0: [1, 2], 1: [3]}.get(h, [])
            for hn in nexts:
                qk8s[(b, hn)] = qk_pool.tile([128, 4, 2, 512], E4, tag="qk", name="qk8t")
                fq.extend(qk_mms(x8s[b], qk8s[(b, hn)], hn))
                add_marker((b, hn))
            if b == 0 and h == 2:
                image_setup(1)
                qk8s[(1, 0)] = qk_pool.tile([128, 4, 2, 512], E4, tag="qk", name="qk8t")
                fq.extend(qk_mms(x8s[1], qk8s[(1, 0)], 0))
                add_marker((1, 0))

            if b == 0 and h == 1:
                # W_out + b_v staging on GPSIMD, well before the bias fold
                for kt in range(8):
                    ws2 = stage_pool.tile([128, 256], F32, tag="wostage")
                    nc.sync.dma_start(out=ws2[:],
                                      in_=wo_d[kt * 128:(kt + 1) * 128, :])
                    nc.gpsimd.tensor_copy(wo8[:, kt, :], ws2[:])
                zscr = stage_pool.tile([128, 16], F32, tag="zscr")
                nc.vector.memset(zscr[:], 0.0)
                nc.gpsimd.tensor_copy(zb[:],
                                      zscr[:].rearrange("p (a b) -> p a b", b=2))
                for kt in range(8):
                    hh, dt = kt // 2, kt % 2
                    nc.gpsimd.tensor_copy(
                        zb[:, kt, 0:1],
                        b_sb[:, hh * 6 + 4 + dt:hh * 6 + 5 + dt])
            if b == 1 and h == 0:
                fq.extend(fold_mms())
            if b == 1 and h == 1:
                fq.extend(outproj_mms(0, cats[0], xbs[0]))

            flush_until((b, h))  # qk8(b,h) drains must be emitted before scores
            qk8 = qk8s[(b, h)]
            e8 = e_pool.tile([128, 8, 1024], E4, tag="e8")
            for isl in range(2):
                for jt in range(8):
                    ps = psS.tile([128, 512], F32, tag="S")
                    nc.tensor.matmul(
                        out=ps[:],
                        lhsT=qk8[:, 2:4, jt // 4, (jt % 4) * 128:(jt % 4 + 1) * 128],
                        rhs=qk8[:, 0:2, isl, :],
                        perf_mode=DR, start=True, stop=True)
                    nc.scalar.activation(e8[:, jt, isl * 512:(isl + 1) * 512],
                                         ps[:], EXP, scale=SCALE,
                                         bias=ebias[:, 0:1])
                    fpop(2)
                # AV of this (h, i-half) becomes filler for what follows
                fq.extend(av_mms(e8, v8s[b], cats[b], h, isl))

        # tail: remaining AV of (b1, h3), then out projection of image 1
        fpop(len(fq))
        for f in outproj_mms(1, cats[1], xbs[1]):
            f()

    nc.compile()
    return nc


_NC = None
_EYE = np.eye(128, dtype=np.float32)


def make_in_maps(x, W_proj, b_proj, W_out, b_out):
    x = np.ascontiguousarray(x, dtype=np.float32).reshape(16, C, N)
    return [
        {
            "x": x[i * B_PER_CORE:(i + 1) * B_PER_CORE],
            "W_proj": np.ascontiguousarray(W_proj, dtype=np.float32),
            "b_proj": np.ascontiguousarray(b_proj, dtype=np.float32),
            "W_out": np.ascontiguousarray(W_out, dtype=np.float32),
            "b_out": np.ascontiguousarray(b_out, dtype=np.float32),
            "eye": _EYE,
        }
        for i in range(N_CORES)
    ]


def kernel(x, W_proj, b_proj, W_out, b_out):
    global _NC
    if _NC is None:
        _NC = _build()
    in_maps = make_in_maps(x, W_proj, b_proj, W_out, b_out)
    res = run_bass_kernel_spmd(_NC, in_maps, core_ids=list(range(N_CORES)))
    out = np.concatenate([res.results[i]["out"] for i in range(N_CORES)], axis=0)
    return out.reshape(16, C, 32, 32)
